# revision 20
# baseline (speedup 1.0000x reference)
"""GQA attention (dense_transformer) distributed over 8 TRN2 NeuronCores.

Sharding: batch (2) x head-groups (4). Core c = 4*b + g handles batch b,
q-heads 4g..4g+3 and kv-head g (GQA group local). Megatron-style:
 - QKV projection with column-sharded weights, x^T replicated per batch group
 - RoPE fused into the PSUM->SBUF eviction (host permutes wq/wk columns to
   [even dims; odd dims] per head so rotation is a partition-block affair)
 - attention computed transposed (scoresT: k on partitions, q on free) so the
   AV matmul needs no transposes; softmax denominators via a ones-matmul
 - attnT reshard via 4 chunked 4-rank AllGathers, each core then projects its
   own quarter of the rows (dynamic-offset DMA selects the quarter) with the
   full wo, writing a (512, 2048) output shard.

All matmuls run as float32r (fp32 with 11-bit mantissa, full PE rate at
free-dim >= 256). Inputs are pre-rounded on host so HW and host agree.
"""

import os
import numpy as np

B = 2
S = 2048
DIM = 2048
NH = 16
NKV = 4
HD = 128
NCORES = 8
QH = NH // NKV  # q heads per core (= per kv group)
SC = 512  # q-chunk / s-chunk size
NSC = S // SC  # 4
NKT = S // HD  # 16 k-tiles
SCALE = 1.0 / float(np.sqrt(HD))
ESHIFT = 12.0  # constant shift inside exp; cancels in softmax
MASKVAL = -1e30

_cache = {}


def _round_f32r(a: np.ndarray) -> np.ndarray:
    """Round float32 to fp32r (11-bit mantissa, RNE) so host/HW agree."""
    u = np.ascontiguousarray(a, dtype=np.float32).view(np.uint32)
    lsb = (u >> 12) & 1
    u = u + 0x7FF + lsb
    u = u & np.uint32(0xFFFFF000)
    return u.view(np.float32)


def _n_ktiles(j: int, causal: bool) -> int:
    return 4 * (j + 1) if causal else NKT


def _build(mode: str):
    """Build + compile the SPMD graph. mode in {'causal', 'none', 'general'}."""
    import concourse.bass as bass
    import concourse.mybir as mybir
    import concourse.tile as tile
    from concourse import bacc
    from concourse.bass import ds
    from concourse.masks import make_identity

    causal = mode == "causal"
    general = mode == "general"
    f32 = mybir.dt.float32
    f32r = mybir.dt.float32r
    bf16 = mybir.dt.bfloat16
    u32 = mybir.dt.uint32

    nc = bacc.Bacc("TRN2", target_bir_lowering=False, debug=False, num_devices=NCORES)

    xt_e = nc.dram_tensor("xt", [DIM, S], f32r, kind="ExternalInput")
    wq_e = nc.dram_tensor("wq", [DIM, QH * HD], f32r, kind="ExternalInput")
    wk_e = nc.dram_tensor("wk", [DIM, HD], f32r, kind="ExternalInput")
    wv_e = nc.dram_tensor("wv", [DIM, HD], f32r, kind="ExternalInput")
    wo_e = nc.dram_tensor("wo", [NH * HD, DIM], bf16, kind="ExternalInput")
    cos_e = nc.dram_tensor("cosT", [HD, S], f32, kind="ExternalInput")
    sin_e = nc.dram_tensor("sinT", [HD, S], f32, kind="ExternalInput")
    ones_e = nc.dram_tensor("ones", [HD, HD], f32r, kind="ExternalInput")
    rsel_e = nc.dram_tensor("rsel", [128, 4], f32, kind="ExternalInput")
    if causal:
        biasd_e = nc.dram_tensor("biasd", [HD, 4 * SC], f32r, kind="ExternalInput")
    if general:
        maskb_e = nc.dram_tensor("maskb", [S, S], f32r, kind="ExternalInput")
    out_e = nc.dram_tensor("out", [SC, DIM], f32, kind="ExternalOutput")

    with tile.TileContext(nc) as tc:
        with (
            tc.tile_pool(name="res", bufs=1) as res,
            tc.tile_pool(name="dram", bufs=1, space="DRAM") as dram,
        ):
            # ---- resident tiles ----
            qT = [res.tile([HD, S], f32r, tag=f"qT{h}", name=f"qT{h}") for h in range(QH)]
            kT = res.tile([HD, S], f32r, tag="kT")
            V = res.tile([HD, S], f32r, tag="V")  # cols [128kc:+128] = V chunk kc
            cosT = res.tile([HD, S], f32, tag="cosT")
            sinT = res.tile([HD, S], f32, tag="sinT")
            ident = res.tile([HD, HD], f32, tag="ident")
            ones = res.tile([HD, HD], f32r, tag="ones")
            ebias = res.tile([128, 1], f32, tag="ebias")
            rsel = res.tile([128, 4], f32, tag="rsel")

            nc.sync.dma_start(out=cosT[:, :], in_=cos_e[:, :])
            nc.sync.dma_start(out=sinT[:, :], in_=sin_e[:, :])
            nc.sync.dma_start(out=ones[:, :], in_=ones_e[:, :])
            nc.sync.dma_start(out=rsel[:, :], in_=rsel_e[:, :])
            make_identity(nc, ident[:, :])
            nc.vector.memset(ebias[:, :], -ESHIFT)

            # selected attnT tiles for this core's out-proj rows (built
            # incrementally after each AllGather: one-hot rsel routes the
            # right chunk; dynamic-offset DMA hangs on this runtime path)
            agt = [res.tile([128, SC], bf16, tag=f"ag{cc}", name=f"ag{cc}") for cc in range(NKT)]

            # bounce buffers for the attnT AllGathers
            bnc_in = dram.tile([4 * SC, SC], bf16)  # rows [512j:+512] = chunk j send
            bnc_out = dram.tile([4 * QH * NKV * HD, SC], bf16)  # (8192, 512)

            # ================= phase 1: QKV projection + RoPE =================
            with (
                tc.tile_pool(name="wqkv", bufs=1) as wp,
                tc.tile_pool(name="xp", bufs=18) as xp,
                tc.tile_pool(name="p1t", bufs=4) as p1t,
                tc.tile_pool(name="vtp", bufs=1) as vtp,
                tc.tile_pool(name="ps1", bufs=4, space="PSUM") as ps1,
                tc.tile_pool(name="ps1t", bufs=2, space="PSUM") as ps1t,
            ):
                # resident weight tiles (lhsT layout: [d-chunk 128, c 128])
                wq_d = [wp.tile([128, QH * HD], f32r, tag=f"wqd{d}", name=f"wqd{d}") for d in range(NKT)]
                wk_t = [wp.tile([128, 128], f32r, tag=f"wk{d}", name=f"wk{d}") for d in range(NKT)]
                wv_t = [wp.tile([128, 128], f32r, tag=f"wv{d}", name=f"wv{d}") for d in range(NKT)]
                vT = vtp.tile([HD, S], f32, tag="vT")

                def rope_evict(psum, dst, sl):
                    """dst[:, sl] = rotate(psum); cosT/sinT are [c;c]/[s;s]
                    stacked. m2s holds the sin product with halves swapped so
                    the combine steps see equal SBUF base partitions (DVE
                    requires SB+SB operands at the same base partition)."""
                    m1 = p1t.tile([128, SC], f32, tag="t1", name="m1")
                    m2s = p1t.tile([128, SC], f32, tag="t2", name="m2s")
                    nc.vector.tensor_mul(m1[:, :], psum[:, :], cosT[:, sl])
                    nc.vector.tensor_mul(m2s[64:128, :], psum[0:64, :], sinT[0:64, sl])
                    nc.vector.tensor_mul(m2s[0:64, :], psum[64:128, :], sinT[64:128, sl])
                    nc.vector.tensor_sub(dst[0:64, sl], m1[0:64, :], m2s[0:64, :])
                    nc.vector.tensor_add(dst[64:128, sl], m1[64:128, :], m2s[64:128, :])

                for sc in range(NSC):
                    sl = slice(SC * sc, SC * sc + SC)
                    xts = [xp.tile([128, SC], f32r, tag="xp", name="xp") for _ in range(NKT)]
                    for d in range(NKT):
                        nc.sync.dma_start(
                            out=xts[d][:, :], in_=xt_e[128 * d : 128 * d + 128, sl]
                        )
                        if sc == 0:
                            nc.sync.dma_start(
                                out=wq_d[d][:, :], in_=wq_e[128 * d : 128 * d + 128, :]
                            )
                            nc.sync.dma_start(
                                out=wk_t[d][:, :], in_=wk_e[128 * d : 128 * d + 128, :]
                            )
                            nc.sync.dma_start(
                                out=wv_t[d][:, :], in_=wv_e[128 * d : 128 * d + 128, :]
                            )
                    for h in range(QH):
                        ps = ps1.tile([128, SC], f32, tag="qkv_ps")
                        for d in range(NKT):
                            nc.tensor.matmul(
                                ps[:, :],
                                lhsT=wq_d[d][:, 128 * h : 128 * h + 128],
                                rhs=xts[d][:, :],
                                start=(d == 0),
                                stop=(d == NKT - 1),
                            )
                        rope_evict(ps, qT[h], sl)
                    ps = ps1.tile([128, SC], f32, tag="qkv_ps")
                    for d in range(NKT):
                        nc.tensor.matmul(
                            ps[:, :],
                            lhsT=wk_t[d][:, :],
                            rhs=xts[d][:, :],
                            start=(d == 0),
                            stop=(d == NKT - 1),
                        )
                    rope_evict(ps, kT, sl)
                    ps = ps1.tile([128, SC], f32, tag="qkv_ps")
                    for d in range(NKT):
                        nc.tensor.matmul(
                            ps[:, :],
                            lhsT=wv_t[d][:, :],
                            rhs=xts[d][:, :],
                            start=(d == 0),
                            stop=(d == NKT - 1),
                        )
                    nc.vector.tensor_copy(vT[:, sl], ps[:, :])

                # transpose vT -> V (kc chunks), f32 transpose then f32r evict
                for kc in range(NKT):
                    cs = slice(128 * kc, 128 * kc + 128)
                    pst = ps1t.tile([128, 128], f32, tag="vtr")
                    nc.tensor.transpose(pst[:, :], vT[:, cs], ident[:, :])
                    nc.vector.tensor_copy(V[:, cs], pst[:, :])

            # ================= phase 2: attention (scoresT) =================
            with (
                tc.tile_pool(name="p2", bufs=12) as p2,
                tc.tile_pool(name="p2b", bufs=1) as p2b,
                tc.tile_pool(name="mb", bufs=4) as mbp,
                tc.tile_pool(name="cnd", bufs=8) as cnd,
                tc.tile_pool(name="pss", bufs=3, space="PSUM") as pss,
                tc.tile_pool(name="psa", bufs=3, space="PSUM") as psa,
                tc.tile_pool(name="psn", bufs=2, space="PSUM") as psn,
            ):
                identr = p2b.tile([HD, HD], f32r, tag="identr")
                nc.vector.tensor_copy(identr[:, :], ident[:, :])
                if causal:
                    biasd = p2b.tile([HD, 4 * SC], f32r, tag="biasd")
                    nc.sync.dma_start(out=biasd[:, :], in_=biasd_e[:, :])

                for jidx, j in enumerate([3, 2, 1, 0]):  # q chunk, big first
                    qsl = slice(SC * j, SC * j + SC)
                    nkt = _n_ktiles(j, causal)
                    for h in range(QH):
                        av_ps = psa.tile([HD, SC], f32, tag="av")
                        sum_ps = psn.tile([128, SC], f32, tag="sums")
                        for kt in range(nkt):
                            ks = slice(128 * kt, 128 * kt + 128)
                            sc_ps = pss.tile([128, SC], f32, tag="sc")
                            has_bias = (causal and kt >= nkt - 4) or general
                            nc.tensor.matmul(
                                sc_ps[:, :],
                                lhsT=kT[:, ks],
                                rhs=qT[h][:, qsl],
                                start=True,
                                stop=not has_bias,
                            )
                            if causal and kt >= nkt - 4:
                                di = kt - (nkt - 4)
                                nc.tensor.matmul(
                                    sc_ps[:, :],
                                    lhsT=identr[:, :],
                                    rhs=biasd[:, SC * di : SC * di + SC],
                                    start=False,
                                    stop=True,
                                )
                            elif general:
                                mb = mbp.tile([128, SC], f32r, tag="mb")
                                nc.sync.dma_start(
                                    out=mb[:, :],
                                    in_=maskb_e[128 * kt : 128 * kt + 128, qsl],
                                )
                                nc.tensor.matmul(
                                    sc_ps[:, :],
                                    lhsT=identr[:, :],
                                    rhs=mb[:, :],
                                    start=False,
                                    stop=True,
                                )
                            e_sb = p2.tile([128, SC], f32r, tag="e")
                            nc.scalar.activation(
                                e_sb[:, :],
                                sc_ps[:, :],
                                mybir.ActivationFunctionType.Exp,
                                bias=ebias[:, :],
                                scale=SCALE,
                            )
                            nc.tensor.matmul(
                                av_ps[:, :],
                                lhsT=V[:, ks],
                                rhs=e_sb[:, :],
                                start=(kt == 0),
                                stop=(kt == nkt - 1),
                            )
                            nc.tensor.matmul(
                                sum_ps[:, :],
                                lhsT=ones[:, :],
                                rhs=e_sb[:, :],
                                start=(kt == 0),
                                stop=(kt == nkt - 1),
                            )
                        rec = p2.tile([128, SC], f32, tag="rec")
                        nc.vector.reciprocal_approx_fast(rec[:, :], sum_ps[:, :])
                        at = p2.tile([HD, SC], bf16, tag="at")
                        nc.vector.tensor_mul(at[:, :], av_ps[:, :], rec[:, :])
                        nc.sync.dma_start(
                            out=bnc_in[SC * j + HD * h : SC * j + HD * h + HD, :],
                            in_=at[:, :],
                        )
                    # AllGather chunk j across the 4 cores of this batch group
                    if os.environ.get("KOPT_NOCC", "0") == "1":
                        nc.sync.dma_start(
                            out=bnc_out[2048 * j : 2048 * j + SC, :],
                            in_=bnc_in[SC * j : SC * j + SC, :],
                        )
                    else:
                        nc.gpsimd.collective_compute(
                            "AllGather",
                            bass.mybir.AluOpType.bypass,
                            replica_groups=[[0, 1, 2, 3], [4, 5, 6, 7]],
                            ins=[bnc_in[SC * j : SC * j + SC, :].opt()],
                            outs=[bnc_out[2048 * j : 2048 * j + 2048, :].opt()],
                        )
                    for cc in range(NKT):
                        cj = cnd.tile([128, SC], bf16, tag="cand", name="cand")
                        nc.sync.dma_start(
                            out=cj[:, :],
                            in_=bnc_out[
                                2048 * j + 128 * cc : 2048 * j + 128 * cc + 128, :
                            ],
                        )
                        if jidx == 0:
                            nc.vector.tensor_scalar_mul(
                                agt[cc][:, :], cj[:, :], rsel[:, j : j + 1]
                            )
                        else:
                            nc.vector.scalar_tensor_tensor(
                                agt[cc][:, :],
                                cj[:, :],
                                rsel[:, j : j + 1],
                                agt[cc][:, :],
                                op0=mybir.AluOpType.mult,
                                op1=mybir.AluOpType.add,
                            )

            # ======== phase 3: out projection of this core's row quarter ========
            with (
                tc.tile_pool(name="wop", bufs=24) as wop,
                tc.tile_pool(name="ps3", bufs=6, space="PSUM") as ps3,
            ):
                for ncc in range(NSC):  # output column chunk
                    nsl = slice(SC * ncc, SC * ncc + SC)
                    pso = [ps3.tile([128, SC], f32, tag="out", name="outps") for _ in range(4)]
                    for cc in range(NKT):
                        wt = wop.tile([128, SC], bf16, tag="wo")
                        nc.sync.dma_start(
                            out=wt[:, :], in_=wo_e[128 * cc : 128 * cc + 128, nsl]
                        )
                        for st in range(4):
                            nc.tensor.matmul(
                                pso[st][:, :],
                                lhsT=agt[cc][:, 128 * st : 128 * st + 128],
                                rhs=wt[:, :],
                                start=(cc == 0),
                                stop=(cc == NKT - 1),
                            )
                    for st in range(4):
                        osb = wop.tile([128, SC], f32, tag="osb", name="osb")
                        nc.scalar.copy(osb[:, :], pso[st][:, :])
                        nc.sync.dma_start(
                            out=out_e[128 * st : 128 * st + 128, nsl],
                            in_=osb[:, :],
                        )

    nc.compile()
    return nc


def _perm_cols(w: np.ndarray, heads: list) -> np.ndarray:
    """Reorder head columns to [even dims; odd dims] for block RoPE."""
    cols = []
    for h in heads:
        base = HD * h
        cols.extend([base + 2 * i for i in range(HD // 2)])
        cols.extend([base + 2 * i + 1 for i in range(HD // 2)])
    return np.ascontiguousarray(w[:, cols])


def kernel(x, wq, wk, wv, wo, freqs_cos, freqs_sin, mask):
    from concourse.bass_utils import run_bass_kernel_spmd

    x = np.asarray(x, dtype=np.float32)
    wq = np.asarray(wq, dtype=np.float32)
    wk = np.asarray(wk, dtype=np.float32)
    wv = np.asarray(wv, dtype=np.float32)
    wo = np.asarray(wo, dtype=np.float32)
    freqs_cos = np.asarray(freqs_cos, dtype=np.float32)
    freqs_sin = np.asarray(freqs_sin, dtype=np.float32)
    mask = np.asarray(mask)

    if not mask.any():
        mode = "none"
    elif np.array_equal(mask, np.triu(np.ones((S, S), dtype=bool), k=1)):
        mode = "causal"
    else:
        mode = "general"

    if mode not in _cache:
        import time as _t

        t0 = _t.time()
        _cache[mode] = _build(mode)
        print(f"[kernel] built mode={mode} in {_t.time() - t0:.1f}s", flush=True)
    nc = _cache[mode]

    # ---- host-side prep (sharding + layout) ----
    xt = [_round_f32r(x[b].T) for b in range(B)]
    import ml_dtypes

    wo_r = wo.astype(ml_dtypes.bfloat16)
    cosT = np.ascontiguousarray(np.concatenate([freqs_cos.T, freqs_cos.T], axis=0))
    sinT = np.ascontiguousarray(np.concatenate([freqs_sin.T, freqs_sin.T], axis=0))
    ones = _round_f32r(np.ones((HD, HD), dtype=np.float32))

    if mode == "causal":
        # 4 diag patterns (delta = 0,128,256,384) packed as (128, 2048):
        # bias[i, 512*di + jq] = MASKVAL if (128*di + i) > jq else 0
        i_ = np.arange(HD)[:, None]
        jq = np.arange(SC)[None, :]
        biasd = np.concatenate(
            [
                np.where(128 * di + i_ > jq, np.float32(MASKVAL), np.float32(0.0))
                for di in range(4)
            ],
            axis=1,
        ).astype(np.float32)
        biasd = _round_f32r(biasd)
    if mode == "general":
        maskb = _round_f32r(
            np.ascontiguousarray(
                np.where(mask.T, np.float32(MASKVAL), np.float32(0.0))
            )
        )

    in_maps = []
    for core in range(NCORES):
        b, g = divmod(core, 4)
        heads = [QH * g + h for h in range(QH)]
        m = {
            "xt": xt[b],
            "wq": _round_f32r(_perm_cols(wq, heads)),
            "wk": _round_f32r(_perm_cols(wk, [g])),
            "wv": _round_f32r(wv[:, HD * g : HD * g + HD]),
            "wo": wo_r,
            "cosT": cosT,
            "sinT": sinT,
            "ones": ones,
            "rsel": np.tile(
                np.eye(4, dtype=np.float32)[g], (128, 1)
            ),
        }
        if mode == "causal":
            m["biasd"] = biasd
        if mode == "general":
            m["maskb"] = maskb
        in_maps.append(m)

    import time as _t

    t0 = _t.time()
    print("[kernel] launching SPMD run", flush=True)
    res = run_bass_kernel_spmd(nc, in_maps, core_ids=list(range(NCORES)))
    print(f"[kernel] SPMD run done in {_t.time() - t0:.1f}s", flush=True)
    kernel._last_result = res

    out = np.empty((B, S, DIM), dtype=np.float32)
    for core in range(NCORES):
        b, g = divmod(core, 4)
        out[b, SC * g : SC * g + SC, :] = res.results[core]["out"]
    return out


# revision 33
# speedup vs baseline: 1.1026x; 1.1026x over previous
"""GQA attention (dense_transformer) distributed over 8 TRN2 NeuronCores.

Sharding: batch (2) x head-groups (4). Core c = 4*b + g handles batch b,
q-heads 4g..4g+3 and kv-head g (GQA group local). Megatron-style:
 - QKV projection with column-sharded weights, x^T replicated per batch group
 - RoPE fused into the PSUM->SBUF eviction (host permutes wq/wk columns to
   [even dims; odd dims] per head so rotation is a partition-block affair)
 - attention computed transposed (scoresT: k on partitions, q on free) so the
   AV matmul needs no transposes; softmax denominators via a ones-matmul
 - attnT reshard via 4 chunked 4-rank AllGathers (small q-chunks first so
   only the last AG is exposed); each core then selects its own row quarter
   with a one-hot DVE multiply (dynamic-offset DMA hangs on this runtime) and
   projects it against the full wo, writing a (512, 2048) output shard.

All matmul operands are bf16 (fp32 PSUM accumulation); softmax runs in fp32
on the scalar engine with a constant shift folded into the exp bias.
Selection passes are interleaved >=2 chunks behind their AllGather so the
in-order DVE stream never blocks on a collective.
"""

import os
import numpy as np

B = 2
S = 2048
DIM = 2048
NH = 16
NKV = 4
HD = 128
NCORES = 8
QH = NH // NKV  # q heads per core (= per kv group)
SC = 512  # q-chunk / s-chunk size
NSC = S // SC  # 4
NKT = S // HD  # 16 k-tiles
SCALE = 1.0 / float(np.sqrt(HD))
ESHIFT = 12.0  # constant shift inside exp; cancels in softmax
MASKVAL = -1e30

_cache = {}


def _round_f32r(a: np.ndarray) -> np.ndarray:
    """Round float32 to fp32r (11-bit mantissa, RNE) so host/HW agree."""
    u = np.ascontiguousarray(a, dtype=np.float32).view(np.uint32)
    lsb = (u >> 12) & 1
    u = u + 0x7FF + lsb
    u = u & np.uint32(0xFFFFF000)
    return u.view(np.float32)


def _n_ktiles(j: int, causal: bool) -> int:
    return 4 * (j + 1) if causal else NKT


def _build(mode: str):
    """Build + compile the SPMD graph. mode in {'causal', 'none', 'general'}."""
    import concourse.bass as bass
    import concourse.mybir as mybir
    import concourse.tile as tile
    from concourse import bacc
    from concourse.bass import ds
    from concourse.masks import make_identity

    causal = mode == "causal"
    general = mode == "general"
    f32 = mybir.dt.float32
    f32r = mybir.dt.float32r
    bf16 = mybir.dt.bfloat16
    u32 = mybir.dt.uint32

    nc = bacc.Bacc("TRN2", target_bir_lowering=False, debug=False, num_devices=NCORES)

    xt_e = nc.dram_tensor("xt", [DIM, S], bf16, kind="ExternalInput")
    wq_e = nc.dram_tensor("wq", [DIM, QH * HD], bf16, kind="ExternalInput")
    wk_e = nc.dram_tensor("wk", [DIM, HD], bf16, kind="ExternalInput")
    wv_e = nc.dram_tensor("wv", [DIM, HD], bf16, kind="ExternalInput")
    wo_e = nc.dram_tensor("wo", [NH * HD, DIM], bf16, kind="ExternalInput")
    cos_e = nc.dram_tensor("cosT", [HD, S], f32, kind="ExternalInput")
    sin_e = nc.dram_tensor("sinT", [HD, S], f32, kind="ExternalInput")
    ones_e = nc.dram_tensor("ones", [HD, HD], bf16, kind="ExternalInput")
    rsel_e = nc.dram_tensor("rsel", [128, 4], f32, kind="ExternalInput")
    if causal:
        biasd_e = nc.dram_tensor("biasd", [HD, 4 * SC], bf16, kind="ExternalInput")
    if general:
        maskb_e = nc.dram_tensor("maskb", [S, S], bf16, kind="ExternalInput")
    out_e = nc.dram_tensor("out", [SC, DIM], f32, kind="ExternalOutput")

    with tile.TileContext(nc) as tc:
        with (
            tc.tile_pool(name="res", bufs=1) as res,
            tc.tile_pool(name="dram", bufs=1, space="DRAM") as dram,
        ):
            # ---- resident tiles ----
            qT = [res.tile([HD, S], bf16, tag=f"qT{h}", name=f"qT{h}") for h in range(QH)]
            kT = res.tile([HD, S], bf16, tag="kT")
            V = res.tile([HD, S], bf16, tag="V")  # cols [128kc:+128] = V chunk kc
            cosT = res.tile([HD, S], f32, tag="cosT")
            sinT = res.tile([HD, S], f32, tag="sinT")
            ident = res.tile([HD, HD], f32, tag="ident")
            ones = res.tile([HD, HD], bf16, tag="ones")
            ebias = res.tile([128, 1], f32, tag="ebias")
            rsel = res.tile([128, 4], f32, tag="rsel")

            nc.sync.dma_start(out=cosT[:, :], in_=cos_e[:, :])
            nc.sync.dma_start(out=sinT[:, :], in_=sin_e[:, :])
            nc.sync.dma_start(out=ones[:, :], in_=ones_e[:, :])
            nc.sync.dma_start(out=rsel[:, :], in_=rsel_e[:, :])
            make_identity(nc, ident[:, :])
            nc.vector.memset(ebias[:, :], -ESHIFT)

            # selected attnT tiles for this core's out-proj rows (built
            # incrementally after each AllGather: one-hot rsel routes the
            # right chunk; dynamic-offset DMA hangs on this runtime path)
            agt = [res.tile([128, SC], bf16, tag=f"ag{cc}", name=f"ag{cc}") for cc in range(NKT)]

            # bounce buffers for the attnT AllGathers
            bnc_in = dram.tile([4 * SC, SC], bf16)  # rows [512j:+512] = chunk j send
            bnc_out = dram.tile([4 * QH * NKV * HD, SC], bf16)  # (8192, 512)

            # ================= phase 1: QKV projection + RoPE =================
            with (
                tc.tile_pool(name="wqkv", bufs=1) as wp,
                tc.tile_pool(name="xp", bufs=18) as xp,
                tc.tile_pool(name="p1t", bufs=4) as p1t,
                tc.tile_pool(name="vtp", bufs=1) as vtp,
                tc.tile_pool(name="ps1", bufs=4, space="PSUM") as ps1,
                tc.tile_pool(name="ps1t", bufs=2, space="PSUM") as ps1t,
            ):
                # resident weight tiles (lhsT layout: [d-chunk 128, c 128])
                wq_d = [wp.tile([128, QH * HD], bf16, tag=f"wqd{d}", name=f"wqd{d}") for d in range(NKT)]
                wk_t = [wp.tile([128, 128], bf16, tag=f"wk{d}", name=f"wk{d}") for d in range(NKT)]
                wv_t = [wp.tile([128, 128], bf16, tag=f"wv{d}", name=f"wv{d}") for d in range(NKT)]
                vT = vtp.tile([HD, S], f32, tag="vT")

                def rope_evict(psum, dst, sl):
                    """dst[:, sl] = rotate(psum); cosT/sinT are [c;c]/[s;s]
                    stacked. m2s holds the sin product with halves swapped so
                    the combine steps see equal SBUF base partitions (DVE
                    requires SB+SB operands at the same base partition)."""
                    m1 = p1t.tile([128, SC], f32, tag="t1", name="m1")
                    m2s = p1t.tile([128, SC], f32, tag="t2", name="m2s")
                    nc.vector.tensor_mul(m1[:, :], psum[:, :], cosT[:, sl])
                    nc.vector.tensor_mul(m2s[64:128, :], psum[0:64, :], sinT[0:64, sl])
                    nc.vector.tensor_mul(m2s[0:64, :], psum[64:128, :], sinT[64:128, sl])
                    nc.vector.tensor_sub(dst[0:64, sl], m1[0:64, :], m2s[0:64, :])
                    nc.vector.tensor_add(dst[64:128, sl], m1[64:128, :], m2s[64:128, :])

                for sc in range(NSC):
                    sl = slice(SC * sc, SC * sc + SC)
                    xts = [xp.tile([128, SC], bf16, tag="xp", name="xp") for _ in range(NKT)]
                    engs = [nc.sync, nc.sync, nc.sync, nc.sync]
                    for d in range(NKT):
                        engs[d % 4].dma_start(
                            out=xts[d][:, :], in_=xt_e[128 * d : 128 * d + 128, sl]
                        )
                        if sc == 0:
                            engs[(d + 1) % 4].dma_start(
                                out=wq_d[d][:, :], in_=wq_e[128 * d : 128 * d + 128, :]
                            )
                            engs[(d + 2) % 4].dma_start(
                                out=wk_t[d][:, :], in_=wk_e[128 * d : 128 * d + 128, :]
                            )
                            engs[(d + 3) % 4].dma_start(
                                out=wv_t[d][:, :], in_=wv_e[128 * d : 128 * d + 128, :]
                            )
                    for h in range(QH):
                        ps = ps1.tile([128, SC], f32, tag="qkv_ps")
                        for d in range(NKT):
                            nc.tensor.matmul(
                                ps[:, :],
                                lhsT=wq_d[d][:, 128 * h : 128 * h + 128],
                                rhs=xts[d][:, :],
                                start=(d == 0),
                                stop=(d == NKT - 1),
                            )
                        rope_evict(ps, qT[h], sl)
                    ps = ps1.tile([128, SC], f32, tag="qkv_ps")
                    for d in range(NKT):
                        nc.tensor.matmul(
                            ps[:, :],
                            lhsT=wk_t[d][:, :],
                            rhs=xts[d][:, :],
                            start=(d == 0),
                            stop=(d == NKT - 1),
                        )
                    rope_evict(ps, kT, sl)
                    ps = ps1.tile([128, SC], f32, tag="qkv_ps")
                    for d in range(NKT):
                        nc.tensor.matmul(
                            ps[:, :],
                            lhsT=wv_t[d][:, :],
                            rhs=xts[d][:, :],
                            start=(d == 0),
                            stop=(d == NKT - 1),
                        )
                    nc.vector.tensor_copy(vT[:, sl], ps[:, :])

                # transpose vT -> V (kc chunks), f32 transpose then f32r evict
                for kc in range(NKT):
                    cs = slice(128 * kc, 128 * kc + 128)
                    pst = ps1t.tile([128, 128], f32, tag="vtr")
                    nc.tensor.transpose(pst[:, :], vT[:, cs], ident[:, :])
                    nc.vector.tensor_copy(V[:, cs], pst[:, :])

            # ================= phase 2: attention (scoresT) =================
            with (
                tc.tile_pool(name="p2", bufs=12) as p2,
                tc.tile_pool(name="p2b", bufs=1) as p2b,
                tc.tile_pool(name="mb", bufs=4) as mbp,
                tc.tile_pool(name="cnd", bufs=8) as cnd,
                tc.tile_pool(name="pss", bufs=3, space="PSUM") as pss,
                tc.tile_pool(name="psa", bufs=3, space="PSUM") as psa,
                tc.tile_pool(name="psn", bufs=2, space="PSUM") as psn,
            ):
                identr = p2b.tile([HD, HD], bf16, tag="identr")
                nc.vector.tensor_copy(identr[:, :], ident[:, :])
                if causal:
                    biasd = p2b.tile([HD, 4 * SC], bf16, tag="biasd")
                    nc.sync.dma_start(out=biasd[:, :], in_=biasd_e[:, :])

                def sel_pass(jidx):
                    # fold AG output chunk jidx into the selected agt tiles;
                    # emitted >=2 chunks after its AllGather was issued so the
                    # in-order DVE stream never waits on the collective
                    for cc in range(NKT):
                        cj = cnd.tile([128, SC], bf16, tag="cand", name="cand")
                        nc.sync.dma_start(
                            out=cj[:, :],
                            in_=bnc_out[
                                2048 * jidx + 128 * cc : 2048 * jidx + 128 * cc + 128,
                                :,
                            ],
                        )
                        if jidx == 0:
                            nc.vector.tensor_scalar_mul(
                                agt[cc][:, :], cj[:, :], rsel[:, jidx : jidx + 1]
                            )
                        else:
                            nc.vector.scalar_tensor_tensor(
                                agt[cc][:, :],
                                cj[:, :],
                                rsel[:, jidx : jidx + 1],
                                agt[cc][:, :],
                                op0=mybir.AluOpType.mult,
                                op1=mybir.AluOpType.add,
                            )

                for jidx, j in enumerate([0, 1, 2, 3]):  # small chunks first:
                    # their AllGathers hide under later attention; only the
                    # last chunk's AG is exposed
                    if jidx >= 2:
                        sel_pass(jidx - 2)
                    qsl = slice(SC * j, SC * j + SC)
                    nkt = _n_ktiles(j, causal)
                    for h in range(QH):
                        av_ps = psa.tile([HD, SC], f32, tag="av")
                        sum_ps = psn.tile([128, SC], f32, tag="sums")
                        for kt in range(nkt):
                            ks = slice(128 * kt, 128 * kt + 128)
                            sc_ps = pss.tile([128, SC], f32, tag="sc")
                            has_bias = (causal and kt >= nkt - 4) or general
                            nc.tensor.matmul(
                                sc_ps[:, :],
                                lhsT=kT[:, ks],
                                rhs=qT[h][:, qsl],
                                start=True,
                                stop=not has_bias,
                            )
                            if causal and kt >= nkt - 4:
                                di = kt - (nkt - 4)
                                nc.tensor.matmul(
                                    sc_ps[:, :],
                                    lhsT=identr[:, :],
                                    rhs=biasd[:, SC * di : SC * di + SC],
                                    start=False,
                                    stop=True,
                                )
                            elif general:
                                mb = mbp.tile([128, SC], bf16, tag="mb")
                                nc.sync.dma_start(
                                    out=mb[:, :],
                                    in_=maskb_e[128 * kt : 128 * kt + 128, qsl],
                                )
                                nc.tensor.matmul(
                                    sc_ps[:, :],
                                    lhsT=identr[:, :],
                                    rhs=mb[:, :],
                                    start=False,
                                    stop=True,
                                )
                            e_sb = p2.tile([128, SC], bf16, tag="e")
                            nc.scalar.activation(
                                e_sb[:, :],
                                sc_ps[:, :],
                                mybir.ActivationFunctionType.Exp,
                                bias=ebias[:, :],
                                scale=SCALE,
                            )
                            nc.tensor.matmul(
                                av_ps[:, :],
                                lhsT=V[:, ks],
                                rhs=e_sb[:, :],
                                start=(kt == 0),
                                stop=(kt == nkt - 1),
                            )
                            nc.tensor.matmul(
                                sum_ps[:, :],
                                lhsT=ones[:, :],
                                rhs=e_sb[:, :],
                                start=(kt == 0),
                                stop=(kt == nkt - 1),
                            )
                        rec = p2.tile([128, SC], f32, tag="rec")
                        nc.vector.reciprocal_approx_fast(rec[:, :], sum_ps[:, :])
                        at = p2.tile([HD, SC], bf16, tag="at")
                        nc.vector.tensor_mul(at[:, :], av_ps[:, :], rec[:, :])
                        nc.sync.dma_start(
                            out=bnc_in[SC * j + HD * h : SC * j + HD * h + HD, :],
                            in_=at[:, :],
                        )
                    # AllGather chunk j across the 4 cores of this batch group
                    if os.environ.get("KOPT_NOCC", "0") == "1":
                        nc.sync.dma_start(
                            out=bnc_out[2048 * j : 2048 * j + SC, :],
                            in_=bnc_in[SC * j : SC * j + SC, :],
                        )
                    else:
                        nc.gpsimd.collective_compute(
                            "AllGather",
                            bass.mybir.AluOpType.bypass,
                            replica_groups=[[0, 1, 2, 3], [4, 5, 6, 7]],
                            ins=[bnc_in[SC * j : SC * j + SC, :].opt()],
                            outs=[bnc_out[2048 * j : 2048 * j + 2048, :].opt()],
                        )

                sel_pass(NSC - 2)
                sel_pass(NSC - 1)

            # ======== phase 3: out projection of this core's row quarter ========
            with (
                tc.tile_pool(name="wop", bufs=24) as wop,
                tc.tile_pool(name="ps3", bufs=6, space="PSUM") as ps3,
            ):
                for ncc in range(NSC):  # output column chunk
                    nsl = slice(SC * ncc, SC * ncc + SC)
                    pso = [ps3.tile([128, SC], f32, tag="out", name="outps") for _ in range(4)]
                    for cc in range(NKT):
                        wt = wop.tile([128, SC], bf16, tag="wo")
                        nc.sync.dma_start(
                            out=wt[:, :], in_=wo_e[128 * cc : 128 * cc + 128, nsl]
                        )
                        for st in range(4):
                            nc.tensor.matmul(
                                pso[st][:, :],
                                lhsT=agt[cc][:, 128 * st : 128 * st + 128],
                                rhs=wt[:, :],
                                start=(cc == 0),
                                stop=(cc == NKT - 1),
                            )
                    for st in range(4):
                        osb = wop.tile([128, SC], f32, tag="osb", name="osb")
                        nc.scalar.copy(osb[:, :], pso[st][:, :])
                        nc.sync.dma_start(
                            out=out_e[128 * st : 128 * st + 128, nsl],
                            in_=osb[:, :],
                        )

    nc.compile()
    return nc


def _perm_cols(w: np.ndarray, heads: list) -> np.ndarray:
    """Reorder head columns to [even dims; odd dims] for block RoPE."""
    cols = []
    for h in heads:
        base = HD * h
        cols.extend([base + 2 * i for i in range(HD // 2)])
        cols.extend([base + 2 * i + 1 for i in range(HD // 2)])
    return np.ascontiguousarray(w[:, cols])


def kernel(x, wq, wk, wv, wo, freqs_cos, freqs_sin, mask):
    from concourse.bass_utils import run_bass_kernel_spmd

    x = np.asarray(x, dtype=np.float32)
    wq = np.asarray(wq, dtype=np.float32)
    wk = np.asarray(wk, dtype=np.float32)
    wv = np.asarray(wv, dtype=np.float32)
    wo = np.asarray(wo, dtype=np.float32)
    freqs_cos = np.asarray(freqs_cos, dtype=np.float32)
    freqs_sin = np.asarray(freqs_sin, dtype=np.float32)
    mask = np.asarray(mask)

    if not mask.any():
        mode = "none"
    elif np.array_equal(mask, np.triu(np.ones((S, S), dtype=bool), k=1)):
        mode = "causal"
    else:
        mode = "general"

    if mode not in _cache:
        import time as _t

        t0 = _t.time()
        _cache[mode] = _build(mode)
        print(f"[kernel] built mode={mode} in {_t.time() - t0:.1f}s", flush=True)
    nc = _cache[mode]

    # ---- host-side prep (sharding + layout) ----
    import ml_dtypes

    xt = [np.ascontiguousarray(x[b].T).astype(ml_dtypes.bfloat16) for b in range(B)]
    wo_r = wo.astype(ml_dtypes.bfloat16)
    cosT = np.ascontiguousarray(np.concatenate([freqs_cos.T, freqs_cos.T], axis=0))
    sinT = np.ascontiguousarray(np.concatenate([freqs_sin.T, freqs_sin.T], axis=0))
    ones = np.ones((HD, HD), dtype=ml_dtypes.bfloat16)

    if mode == "causal":
        # 4 diag patterns (delta = 0,128,256,384) packed as (128, 2048):
        # bias[i, 512*di + jq] = MASKVAL if (128*di + i) > jq else 0
        i_ = np.arange(HD)[:, None]
        jq = np.arange(SC)[None, :]
        biasd = np.concatenate(
            [
                np.where(128 * di + i_ > jq, np.float32(MASKVAL), np.float32(0.0))
                for di in range(4)
            ],
            axis=1,
        ).astype(ml_dtypes.bfloat16)
    if mode == "general":
        maskb = np.ascontiguousarray(
            np.where(mask.T, np.float32(MASKVAL), np.float32(0.0))
        ).astype(ml_dtypes.bfloat16)

    in_maps = []
    for core in range(NCORES):
        b, g = divmod(core, 4)
        heads = [QH * g + h for h in range(QH)]
        m = {
            "xt": xt[b],
            "wq": _perm_cols(wq, heads).astype(ml_dtypes.bfloat16),
            "wk": _perm_cols(wk, [g]).astype(ml_dtypes.bfloat16),
            "wv": np.ascontiguousarray(wv[:, HD * g : HD * g + HD]).astype(ml_dtypes.bfloat16),
            "wo": wo_r,
            "cosT": cosT,
            "sinT": sinT,
            "ones": ones,
            "rsel": np.tile(
                np.eye(4, dtype=np.float32)[g], (128, 1)
            ),
        }
        if mode == "causal":
            m["biasd"] = biasd
        if mode == "general":
            m["maskb"] = maskb
        in_maps.append(m)

    import time as _t

    t0 = _t.time()
    print("[kernel] launching SPMD run", flush=True)
    res = run_bass_kernel_spmd(nc, in_maps, core_ids=list(range(NCORES)))
    print(f"[kernel] SPMD run done in {_t.time() - t0:.1f}s", flush=True)
    kernel._last_result = res

    out = np.empty((B, S, DIM), dtype=np.float32)
    for core in range(NCORES):
        b, g = divmod(core, 4)
        out[b, SC * g : SC * g + SC, :] = res.results[core]["out"]
    return out
